# revision 10
# baseline (speedup 1.0000x reference)
"""Trainium2 Bass kernel for nn_CrossAttention (B=2, T=V=4096, 16 heads, d=64).

Math: the reference einsums contract the k/v group axis g, so
  weight = softmax((x@Wq) @ (adj @ sum_g Wk_g)^T / sqrt(64))
  out    = (weight @ (adj @ sum_g Wv_g)) @ Wo + bo
The group fold (sum over g of Wk/Wv columns) is done host-side.

Sharding: 8 cores = (batch b, quarter of T).  Each core takes t-rows
[tq*1024, (tq+1)*1024) of batch b, uses adj[b] (redundant across the 4
cores of the same b), writes its own out slice.  No collectives.

v2 design (vs v1 at 880us):
- fp16 datapath: PE moving-operand streams at 1 col/cycle fp16 vs 0.5
  fp32 -> all big matmuls 2x.  Host casts x/adj/weights to fp16.
- S^T matmuls row-tiled: K=64 pairs at tile_position (0,0)/(64,0) run
  concurrently (2 heads per 512-cycle slot).  kT keeps a duplicated
  copy of K^T in partitions 64..127 (via duplicated Wk columns), qT
  stores head pairs stacked (even head in parts 0..63, odd in 64..127)
  which is exactly the Q-projection PSUM layout -> no re-shuffle.
- softmax exp split ACT/DVE: ACT does ~2/3 of the [128,1024] S tiles
  (native Exp), DVE does the rest via two custom DVE ops
  (cubic seed p ~ exp(s/256), then 5 squarings -> exp(s/8); rel err
  ~1e-3 which is fine for the 2e-2 gate).  This breaks the v1
  ACT-only bottleneck (552us of Exp).
- transposes via LDWEIGHTS+matmul with fp16 identity moving operand
  (56ns vs 275ns is_transpose), PSUM evacuations batched to [128,1024]
  with fused bias+fp16-cast on ACT.
"""

import numpy as np

import concourse.bass as bass
import concourse.tile as tile
from concourse import bacc, mybir
from concourse.masks import make_identity

F32 = mybir.dt.float32
F16 = mybir.dt.float16

# Problem constants (hardcoded per the harness contract).
B = 2
T = 4096
V = 4096
E = 1024     # n_embd
HID = 1024   # n_hidden
NH = 16
DH = 64
G = 4
N_CORES = 8
T_CORE = (B * T) // N_CORES  # 1024 t-rows per core
P = 128

T_TILE = 512          # t-columns per attention tile
HPG = 4               # heads per (tt,hg) group
SCALE = 1.0 / 8.0     # 1/sqrt(DH)

# DVE exp: p(s) = 1 + c1 s + c2 s^2 + c3 s^3 ~ exp(s/256) on s in [-72,72],
# then p^32.  Max rel err ~1.1e-3 (measured on HW).
EXP_C1 = 0.0039065834815540865
EXP_C2 = 7.670921957387487e-06
EXP_C3 = 9.875269664161357e-09

# Of each (tt,hg) group's 64 [128,1024] exp tiles, this many go to the DVE
# (rest to ACT).  ACT 1146ns/tile vs DVE 2x1219ns -> balance ~19/64 after
# accounting for the normalize work that also lands on the DVE.
DVE_EXP_SHARE = 19
# PV matmuls for a tile are emitted this many pair-slots later, so the
# exp latency (up to 2.4us on the DVE path) never blocks the in-order
# PE queue between S matmuls.
PV_LAG = 4


def _register_exp_ops():
    """Register the two custom DVE ops (idempotent).  Appending to
    concourse.dve_ops.OPS is the documented authoring path; the uop table
    ships per-NEFF so no firmware change is involved."""
    from concourse import dve_ops as dops
    from concourse.dve_spec import Spec, Src0, One, sq, lower
    from concourse.dve_uop import DveOpSpec

    existing = {op.name: op for op in dops.OPS}
    if "EXP_SEED_ANT" in existing:
        return existing["EXP_SEED_ANT"], existing["EXP_SQ5_ANT"]

    def ref_seed(in0, in1, s0, s1, imm2):
        x = in0.astype(np.float32)
        return ((np.float32(s0) * x + np.float32(s1)) * x
                + np.float32(imm2)) * x + np.float32(1.0)

    def ref_sq5(in0, in1, s0, s1, imm2):
        x = in0.astype(np.float64)
        for _ in range(5):
            x = x * x
        return x.astype(np.float32)

    spec_seed = Spec(
        body=((Src0 * dops.C0 + dops.C1) * Src0 + dops.C2) * Src0 + One,
        reference=ref_seed,
    )
    spec_sq5 = Spec(body=sq(sq(sq(sq(sq(Src0))))), reference=ref_sq5)

    out = []
    for name, spec in (("EXP_SEED_ANT", spec_seed), ("EXP_SQ5_ANT", spec_sq5)):
        row = max(dops._SUB_OPCODE_FOR_NAME.values()) + 1
        dops._SUB_OPCODE_FOR_NAME[name] = row
        shas = {}
        for ver in ("v3", "v4"):
            dspec = DveOpSpec(name=name, opcode=row,
                              uops=lower(spec, ver=ver), rd1_en=False)
            shas[ver] = dspec.sha(ver)
        op = dops.DveOp(name, spec, subdim=False, uops_sha=shas)
        dops.OPS.append(op)
        dops.CUSTOM_DVE_SPECS[name] = spec
        out.append(op)
    return out


def build_nc():
    OP_SEED, OP_SQ5 = _register_exp_ops()

    EB = E // P                # 8  e-blocks
    DB = HID // P              # 8  dual-head blocks
    NVB = V // P               # 32 v-blocks
    NTT = T_CORE // T_TILE     # 2  t-halves
    NHG = NH // HPG            # 4  head groups
    NCH_V = V // P             # 32 adj chunks of 128 rows
    NCH_T = T_CORE // P        # 8  x chunks

    nc = bacc.Bacc("TRN2", target_bir_lowering=False, debug=False,
                   num_devices=N_CORES)

    x_sl = nc.declare_dram_parameter("x_sl", [T_CORE, E], F16, isOutput=False)
    adj_b = nc.declare_dram_parameter("adj_b", [V, E], F16, isOutput=False)
    Wq_d = nc.declare_dram_parameter("Wq_d", [E, HID], F16, isOutput=False)
    bq_d = nc.declare_dram_parameter("bq_d", [P, DB], F32, isOutput=False)
    Wk_d = nc.declare_dram_parameter("Wk_d", [E, P], F16, isOutput=False)
    bk_d = nc.declare_dram_parameter("bk_d", [P], F32, isOutput=False)
    Wv_d = nc.declare_dram_parameter("Wv_d", [E, DH], F16, isOutput=False)
    bv_d = nc.declare_dram_parameter("bv_d", [DH], F32, isOutput=False)
    Wo_d = nc.declare_dram_parameter("Wo_d", [HID, HID], F16, isOutput=False)
    bo_d = nc.declare_dram_parameter("bo_d", [HID], F32, isOutput=False)
    out_sl = nc.declare_dram_parameter("out_sl", [T_CORE, HID], F32,
                                       isOutput=True)
    sums_dram = nc.dram_tensor("sums_scratch", [NH, T_CORE], F32)

    def bcast_ap(param, n_part):
        a = param[:] if not isinstance(param, bass.AP) else param
        return bass.AP(tensor=a.tensor, offset=a.offset,
                       ap=[[0, n_part]] + list(a.ap))

    from contextlib import ExitStack
    with tile.TileContext(nc, pool_alloc_mode="queue") as tc, ExitStack() as st:
        consts = st.enter_context(tc.tile_pool(name="consts", bufs=1))
        persist = st.enter_context(tc.tile_pool(name="persist", bufs=1))

        ident32 = consts.tile([P, P], F32)
        make_identity(nc, ident32[:])
        ident = consts.tile([P, P], F16)
        nc.vector.tensor_copy(ident[:], ident32[:])
        bq_sb = consts.tile([P, DB], F32)
        nc.sync.dma_start(bq_sb[:], bq_d[:])
        bk_sb = consts.tile([P, 1], F32)
        nc.sync.dma_start(bk_sb[:], bk_d.rearrange("(a one) -> a one", one=1))
        bvb = consts.tile([P, DH], F32)
        nc.gpsimd.dma_start(bvb[:], bcast_ap(bv_d, P))
        bob = consts.tile([P, HID], F32)
        nc.gpsimd.dma_start(bob[:], bcast_ap(bo_d, P))

        # Persistent fp16 operands.
        kT = persist.tile([P, V], F16)             # rows 0-63 K^T, 64-127 dup
        vt = persist.tile([P, NVB, DH + 1], F16)   # V~ + ones col
        qT = persist.tile([P, DB, T_CORE], F16)    # dual-head: even head in
        attnT = persist.tile([P, DB, T_CORE], F16) # parts 0-63, odd in 64-127
        nc.gpsimd.memset(vt[:, :, DH:DH + 1], 1.0)

        # ---- Phase B: K^T and V~ from adj ----
        with (
            tc.tile_pool(name="bin", bufs=3) as bin_p,
            tc.tile_pool(name="baT", bufs=2) as baT_p,
            tc.tile_pool(name="bpsA", bufs=2, space="PSUM") as bpsA,
            tc.tile_pool(name="bpsK", bufs=1, space="PSUM") as bpsK,
            tc.tile_pool(name="bpsV", bufs=1, space="PSUM") as bpsV,
            tc.tile_pool(name="bw1", bufs=1) as bw1,
        ):
            Wk_sb = bw1.tile([P, EB, P], F16)
            nc.sync.dma_start(Wk_sb[:],
                              Wk_d.rearrange("(eb ep) d -> ep eb d", ep=P))
            Wv_sb = bw1.tile([P, EB, DH], F16)
            nc.sync.dma_start(Wv_sb[:],
                              Wv_d.rearrange("(eb ep) d -> ep eb d", ep=P))

            KGRP = 8  # chunks whose K/V projections accumulate before evac
            SUB = 4   # chunks per K-proj batch (N=512 matmuls)
            for cg in range(NCH_V // KGRP):
                pk = bpsK.tile([P, KGRP * P], F32, tag="pk")
                pv = bpsV.tile([P, KGRP, DH], F32, tag="pv")
                for sub in range(KGRP // SUB):
                    aT4 = baT_p.tile([P, EB, SUB * P], F16, tag="aT4")
                    for ci4 in range(SUB):
                        ci = sub * SUB + ci4
                        r0 = (cg * KGRP + ci) * P
                        adj_in = bin_p.tile([P, E], F16, tag="adj_in")
                        nc.sync.dma_start(adj_in[:], adj_b[r0:r0 + P, :])
                        psA = bpsA.tile([P, EB, P], F32, tag="psA")
                        for eb in range(EB):
                            nc.tensor.matmul(psA[:, eb, :],
                                             adj_in[:, eb * P:(eb + 1) * P],
                                             ident[:], start=True, stop=True)
                        nc.scalar.activation(
                            aT4[:, :, ci4 * P:(ci4 + 1) * P], psA[:],
                            mybir.ActivationFunctionType.Copy)
                    for eb in range(EB):
                        nc.tensor.matmul(
                            pk[:, sub * SUB * P:(sub + 1) * SUB * P],
                            Wk_sb[:, eb, :], aT4[:, eb, :],
                            start=(eb == 0), stop=(eb == EB - 1))
                    for ci4 in range(SUB):
                        ci = sub * SUB + ci4
                        for eb in range(EB):
                            nc.tensor.matmul(
                                pv[:, ci, :],
                                aT4[:, eb, ci4 * P:(ci4 + 1) * P],
                                Wv_sb[:, eb, :],
                                start=(eb == 0), stop=(eb == EB - 1))
                # evac: kT with bias (per-partition, fp16 cast on ACT)
                nc.scalar.activation(kT[:, cg * KGRP * P:(cg + 1) * KGRP * P],
                                     pk[:],
                                     mybir.ActivationFunctionType.Identity,
                                     bias=bk_sb[:])
                # evac: V~ with bias (free-dim bias -> DVE tensor add)
                bvb_b = bass.AP(tensor=bvb[:].tensor, offset=bvb[:].offset,
                                ap=[list(bvb[:].ap[0]), [0, KGRP],
                                    list(bvb[:].ap[1])])
                nc.vector.tensor_add(vt[:, cg * KGRP:(cg + 1) * KGRP, 0:DH],
                                     pv[:], bvb_b)

        # ---- Phase C: q^T from x (dual-head layout comes free) ----
        with (
            tc.tile_pool(name="cin", bufs=3) as cin_p,
            tc.tile_pool(name="cxT", bufs=1) as cxT_p,
            tc.tile_pool(name="cpsA", bufs=2, space="PSUM") as cpsA,
            tc.tile_pool(name="cpsQ", bufs=2, space="PSUM") as cpsQ,
            tc.tile_pool(name="cw1", bufs=1) as cw1,
        ):
            Wq_sb = cw1.tile([P, EB, HID], F16)
            nc.sync.dma_start(Wq_sb[:],
                              Wq_d.rearrange("(eb ep) d -> ep eb d", ep=P))
            xT = cxT_p.tile([P, EB, T_CORE], F16)
            for ch in range(NCH_T):
                r0 = ch * P
                x_in = cin_p.tile([P, E], F16, tag="x_in")
                nc.sync.dma_start(x_in[:], x_sl[r0:r0 + P, :])
                psA = cpsA.tile([P, EB, P], F32, tag="psA")
                for eb in range(EB):
                    nc.tensor.matmul(psA[:, eb, :],
                                     x_in[:, eb * P:(eb + 1) * P],
                                     ident[:], start=True, stop=True)
                nc.scalar.activation(xT[:, :, r0:r0 + P], psA[:],
                                     mybir.ActivationFunctionType.Copy)
            for tt in range(NTT):
                ts0 = tt * T_TILE
                for db in range(DB):
                    pq = cpsQ.tile([P, T_TILE], F32, tag="pq")
                    for eb in range(EB):
                        nc.tensor.matmul(
                            pq[:], Wq_sb[:, eb, db * P:(db + 1) * P],
                            xT[:, eb, ts0:ts0 + T_TILE],
                            start=(eb == 0), stop=(eb == EB - 1))
                    nc.scalar.activation(qT[:, db, ts0:ts0 + T_TILE], pq[:],
                                         mybir.ActivationFunctionType.Identity,
                                         bias=bq_sb[:, db:db + 1])

        # ---- Phase D: attention ----
        with (
            tc.tile_pool(name="dP", bufs=7) as dP,
            tc.tile_pool(name="dtmp", bufs=3) as dtmp,
            tc.tile_pool(name="dnorm", bufs=1) as dn,
            tc.tile_pool(name="dpsum", bufs=2, space="PSUM") as dps,
            tc.tile_pool(name="opsum", bufs=1, space="PSUM") as ops,
        ):
            def norm_copy(state):
                O4t, onorm, tt, hg = state
                nc.scalar.activation(onorm[:], O4t[:],
                                     mybir.ActivationFunctionType.Copy)

            def norm_rest(state):
                O4t, onorm, tt, hg = state
                ts0 = tt * T_TILE
                j0 = hg * (HPG // 2)
                h0 = hg * HPG
                nc.gpsimd.dma_start(
                    sums_dram[h0:h0 + HPG, ts0:ts0 + T_TILE],
                    onorm[DH:DH + 1, :, :])
                sbc = dn.tile([DH, HPG, T_TILE], F32, tag="sbc")
                nc.gpsimd.dma_start(
                    sbc[:],
                    bcast_ap(sums_dram[h0:h0 + HPG, ts0:ts0 + T_TILE], DH))
                rec = dn.tile([DH, HPG, T_TILE], F32, tag="rec")
                nc.vector.reciprocal_approx_fast(rec[:], sbc[:])
                for pp in range(HPG // 2):
                    j = j0 + pp
                    nc.vector.tensor_mul(
                        attnT[0:DH, j, ts0:ts0 + T_TILE],
                        onorm[0:DH, 2 * pp, :], rec[:, 2 * pp, :])
                    nrm = dn.tile([DH, T_TILE], F16, tag="nrm%d" % pp)
                    nc.gpsimd.tensor_mul(nrm[:], onorm[0:DH, 2 * pp + 1, :],
                                         rec[:, 2 * pp + 1, :])
                    nc.gpsimd.dma_start(
                        attnT[DH:P, j, ts0:ts0 + T_TILE], nrm[:])

            prev = None
            for tt in range(NTT):
                ts0 = tt * T_TILE
                for hg in range(NHG):
                    j0 = hg * (HPG // 2)   # dual-head slots j0, j0+1
                    O4t = ops.tile([DH + 1, HPG, T_TILE], F32, tag="O4t")
                    onorm = dn.tile([DH + 1, HPG, T_TILE], F32,
                                    tag="onorm")
                    exp_i = 0
                    pend = []

                    def emit_pv(item, O4t=O4t):
                        P2, vb, pp = item
                        for h2 in range(2):
                            nc.tensor.matmul(
                                O4t[:, 2 * pp + h2, :], vt[:, vb, :],
                                P2[:, h2 * T_TILE:(h2 + 1) * T_TILE],
                                start=(vb == 0), stop=(vb == NVB - 1),
                                skip_group_check=True)

                    for vb in range(NVB):
                        for pp in range(HPG // 2):
                            j = j0 + pp
                            S2 = dps.tile([P, 2 * T_TILE], F32, tag="S2")
                            nc.tensor.matmul(
                                S2[:, 0:T_TILE],
                                kT[0:DH, vb * P:(vb + 1) * P],
                                qT[0:DH, j, ts0:ts0 + T_TILE],
                                start=True, stop=True, tile_position=(0, 0))
                            nc.tensor.matmul(
                                S2[:, T_TILE:2 * T_TILE],
                                kT[DH:P, vb * P:(vb + 1) * P],
                                qT[DH:P, j, ts0:ts0 + T_TILE],
                                start=True, stop=True, tile_position=(64, 0))
                            P2 = dP.tile([P, 2 * T_TILE], F16, tag="P2")
                            use_dve = (exp_i * DVE_EXP_SHARE) // 64 != \
                                      ((exp_i + 1) * DVE_EXP_SHARE) // 64
                            if use_dve:
                                Ptmp = dtmp.tile([P, 2 * T_TILE], F32,
                                                 tag="Ptmp")
                                nc.vector._custom_dve(
                                    OP_SEED, out=Ptmp[:], in0=S2[:],
                                    s0=EXP_C3, s1=EXP_C2, imm2=EXP_C1)
                                nc.vector._custom_dve(
                                    OP_SQ5, out=P2[:], in0=Ptmp[:])
                            else:
                                nc.scalar.activation(
                                    P2[:], S2[:],
                                    mybir.ActivationFunctionType.Exp,
                                    scale=SCALE)
                            pend.append((P2, vb, pp))
                            exp_i += 1
                            # lagged normalize of the previous group: the O
                            # evacuation must land before this group's first
                            # PV write (slot PV_LAG+1); the rest is pure
                            # latency hiding.
                            if exp_i == 2 and prev is not None:
                                norm_copy(prev)
                            if exp_i == 8 and prev is not None:
                                norm_rest(prev)
                                prev = None
                            while len(pend) > PV_LAG:
                                emit_pv(pend.pop(0))
                    for item in pend:
                        emit_pv(item)
                    prev = (O4t, onorm, tt, hg)
            norm_copy(prev)
            norm_rest(prev)

        # ---- Phase E: output projection ----
        with (
            tc.tile_pool(name="ework", bufs=3) as ew,
            tc.tile_pool(name="ew1", bufs=1) as ew1,
            tc.tile_pool(name="epsum", bufs=2, space="PSUM") as ep,
        ):
            Wo_sb = ew1.tile([P, DB, HID], F16)
            nc.sync.dma_start(Wo_sb[:],
                              Wo_d.rearrange("(kb kp) e -> kp kb e", kp=P))
            for tc_i in range(T_CORE // P):
                for eh in range(HID // T_TILE):
                    po = ep.tile([P, T_TILE], F32, tag="po")
                    for kb in range(DB):
                        nc.tensor.matmul(
                            po[:], attnT[:, kb, tc_i * P:(tc_i + 1) * P],
                            Wo_sb[:, kb, eh * T_TILE:(eh + 1) * T_TILE],
                            start=(kb == 0), stop=(kb == DB - 1))
                    ot = ew.tile([P, T_TILE], F32, tag="ot")
                    nc.vector.tensor_add(
                        ot[:], po[:], bob[:, eh * T_TILE:(eh + 1) * T_TILE])
                    nc.sync.dma_start(
                        out_sl[tc_i * P:(tc_i + 1) * P,
                               eh * T_TILE:(eh + 1) * T_TILE], ot[:])

    nc.compile()
    return nc


_NC = None


def _get_nc():
    global _NC
    if _NC is None:
        _NC = build_nc()
    return _NC


def _make_in_maps(inputs):
    x = np.asarray(inputs["x"], np.float32)
    adj = np.asarray(inputs["adj"], np.float32)
    Wq = np.asarray(inputs["Wq"], np.float32)
    bq = np.asarray(inputs["bq"], np.float32)
    Wk_f = np.asarray(inputs["Wk"], np.float32).reshape(E, G, DH).sum(axis=1)
    bk_f = np.asarray(inputs["bk"], np.float32).reshape(G, DH).sum(axis=0)
    Wv_f = np.asarray(inputs["Wv"], np.float32).reshape(E, G, DH).sum(axis=1)
    bv_f = np.asarray(inputs["bv"], np.float32).reshape(G, DH).sum(axis=0)
    Wo = np.asarray(inputs["Wo"], np.float32)
    bo = np.asarray(inputs["bo"], np.float32)

    Wk_dup = np.ascontiguousarray(
        np.concatenate([Wk_f, Wk_f], axis=1).astype(np.float16))  # [E,128]
    bk_dup = np.ascontiguousarray(np.concatenate([bk_f, bk_f]).astype(np.float32))
    bq_dbl = np.ascontiguousarray(bq.reshape(HID // P, P).T.astype(np.float32))

    Wq_16 = np.ascontiguousarray(Wq.astype(np.float16))
    Wv_16 = np.ascontiguousarray(Wv_f.astype(np.float16))
    Wo_16 = np.ascontiguousarray(Wo.astype(np.float16))
    x16 = x.astype(np.float16)
    adj16 = adj.astype(np.float16)

    in_maps = []
    for c in range(N_CORES):
        b = c // (N_CORES // B)
        tq = c % (N_CORES // B)
        in_maps.append({
            "x_sl": np.ascontiguousarray(
                x16[b, tq * T_CORE:(tq + 1) * T_CORE, :]),
            "adj_b": np.ascontiguousarray(adj16[b]),
            "Wq_d": Wq_16, "bq_d": bq_dbl,
            "Wk_d": Wk_dup, "bk_d": bk_dup,
            "Wv_d": Wv_16, "bv_d": np.ascontiguousarray(bv_f),
            "Wo_d": Wo_16, "bo_d": np.ascontiguousarray(bo),
        })
    return in_maps


def kernel(x, adj, Wq, bq, Wk, bk, Wv, bv, Wo, bo):
    inputs = dict(x=x, adj=adj, Wq=Wq, bq=bq, Wk=Wk, bk=bk, Wv=Wv, bv=bv,
                  Wo=Wo, bo=bo)
    nc = _get_nc()
    in_maps = _make_in_maps(inputs)

    from concourse.bass_utils import run_bass_kernel_spmd
    res = run_bass_kernel_spmd(nc, in_maps, list(range(N_CORES)))

    out = np.empty((B, T, HID), np.float32)
    for c in range(N_CORES):
        b = c // (N_CORES // B)
        tq = c % (N_CORES // B)
        out[b, tq * T_CORE:(tq + 1) * T_CORE, :] = res.results[c]["out_sl"]
    return out


# revision 12
# speedup vs baseline: 1.0132x; 1.0132x over previous
"""Trainium2 Bass kernel for nn_CrossAttention (B=2, T=V=4096, 16 heads, d=64).

Math: the reference einsums contract the k/v group axis g, so
  weight = softmax((x@Wq) @ (adj @ sum_g Wk_g)^T / sqrt(64))
  out    = (weight @ (adj @ sum_g Wv_g)) @ Wo + bo
The group fold (sum over g of Wk/Wv columns) is done host-side.

Sharding: 8 cores = (batch b, quarter of T).  Each core takes t-rows
[tq*1024, (tq+1)*1024) of batch b, uses adj[b] (redundant across the 4
cores of the same b), writes its own out slice.  No collectives.

v2 design (vs v1 at 880us):
- fp16 datapath: PE moving-operand streams at 1 col/cycle fp16 vs 0.5
  fp32 -> all big matmuls 2x.  Host casts x/adj/weights to fp16.
- S^T matmuls row-tiled: K=64 pairs at tile_position (0,0)/(64,0) run
  concurrently (2 heads per 512-cycle slot).  kT keeps a duplicated
  copy of K^T in partitions 64..127 (via duplicated Wk columns), qT
  stores head pairs stacked (even head in parts 0..63, odd in 64..127)
  which is exactly the Q-projection PSUM layout -> no re-shuffle.
- softmax exp split ACT/DVE: ACT does ~2/3 of the [128,1024] S tiles
  (native Exp), DVE does the rest via two custom DVE ops
  (cubic seed p ~ exp(s/256), then 5 squarings -> exp(s/8); rel err
  ~1e-3 which is fine for the 2e-2 gate).  This breaks the v1
  ACT-only bottleneck (552us of Exp).
- transposes via LDWEIGHTS+matmul with fp16 identity moving operand
  (56ns vs 275ns is_transpose), PSUM evacuations batched to [128,1024]
  with fused bias+fp16-cast on ACT.
"""

import numpy as np

import concourse.bass as bass
import concourse.tile as tile
from concourse import bacc, mybir
from concourse.masks import make_identity

F32 = mybir.dt.float32
F16 = mybir.dt.float16

# Problem constants (hardcoded per the harness contract).
B = 2
T = 4096
V = 4096
E = 1024     # n_embd
HID = 1024   # n_hidden
NH = 16
DH = 64
G = 4
N_CORES = 8
T_CORE = (B * T) // N_CORES  # 1024 t-rows per core
P = 128

T_TILE = 512          # t-columns per attention tile
HPG = 4               # heads per (tt,hg) group
SCALE = 1.0 / 8.0     # 1/sqrt(DH)

# DVE exp: p(s) = 1 + c1 s + c2 s^2 + c3 s^3 ~ exp(s/256) on s in [-72,72],
# then p^32.  Max rel err ~1.1e-3 (measured on HW).
EXP_C1 = 0.0039065834815540865
EXP_C2 = 7.670921957387487e-06
EXP_C3 = 9.875269664161357e-09

# Of each (tt,hg) group's 64 [128,1024] exp tiles, this many go to the DVE
# (rest to ACT).  ACT 1146ns/tile vs DVE 2x1219ns -> balance ~19/64 after
# accounting for the normalize work that also lands on the DVE.
DVE_EXP_SHARE = 19
# PV matmuls for a tile are emitted this many pair-slots later, so the
# exp latency (up to 2.4us on the DVE path) never blocks the in-order
# PE queue between S matmuls.
PV_LAG = 4


def _register_exp_ops():
    """Register the two custom DVE ops (idempotent).  Appending to
    concourse.dve_ops.OPS is the documented authoring path; the uop table
    ships per-NEFF so no firmware change is involved."""
    from concourse import dve_ops as dops
    from concourse.dve_spec import Spec, Src0, One, sq, lower
    from concourse.dve_uop import DveOpSpec

    existing = {op.name: op for op in dops.OPS}
    if "EXP_SEED_ANT" in existing:
        return existing["EXP_SEED_ANT"], existing["EXP_SQ5_ANT"]

    def ref_seed(in0, in1, s0, s1, imm2):
        x = in0.astype(np.float32)
        return ((np.float32(s0) * x + np.float32(s1)) * x
                + np.float32(imm2)) * x + np.float32(1.0)

    def ref_sq5(in0, in1, s0, s1, imm2):
        x = in0.astype(np.float64)
        for _ in range(5):
            x = x * x
        return x.astype(np.float32)

    spec_seed = Spec(
        body=((Src0 * dops.C0 + dops.C1) * Src0 + dops.C2) * Src0 + One,
        reference=ref_seed,
    )
    spec_sq5 = Spec(body=sq(sq(sq(sq(sq(Src0))))), reference=ref_sq5)

    out = []
    for name, spec in (("EXP_SEED_ANT", spec_seed), ("EXP_SQ5_ANT", spec_sq5)):
        row = max(dops._SUB_OPCODE_FOR_NAME.values()) + 1
        dops._SUB_OPCODE_FOR_NAME[name] = row
        shas = {}
        for ver in ("v3", "v4"):
            dspec = DveOpSpec(name=name, opcode=row,
                              uops=lower(spec, ver=ver), rd1_en=False)
            shas[ver] = dspec.sha(ver)
        op = dops.DveOp(name, spec, subdim=False, uops_sha=shas)
        dops.OPS.append(op)
        dops.CUSTOM_DVE_SPECS[name] = spec
        out.append(op)
    return out


def build_nc():
    OP_SEED, OP_SQ5 = _register_exp_ops()

    EB = E // P                # 8  e-blocks
    DB = HID // P              # 8  dual-head blocks
    NVB = V // P               # 32 v-blocks
    NTT = T_CORE // T_TILE     # 2  t-halves
    NHG = NH // HPG            # 4  head groups
    NCH_V = V // P             # 32 adj chunks of 128 rows
    NCH_T = T_CORE // P        # 8  x chunks

    nc = bacc.Bacc("TRN2", target_bir_lowering=False, debug=False,
                   num_devices=N_CORES)

    x_sl = nc.declare_dram_parameter("x_sl", [T_CORE, E], F16, isOutput=False)
    adj_b = nc.declare_dram_parameter("adj_b", [V, E], F16, isOutput=False)
    Wq_d = nc.declare_dram_parameter("Wq_d", [E, HID], F16, isOutput=False)
    bq_d = nc.declare_dram_parameter("bq_d", [P, DB], F32, isOutput=False)
    Wk_d = nc.declare_dram_parameter("Wk_d", [E, P], F16, isOutput=False)
    bk_d = nc.declare_dram_parameter("bk_d", [P], F32, isOutput=False)
    Wv_d = nc.declare_dram_parameter("Wv_d", [E, DH], F16, isOutput=False)
    bv_d = nc.declare_dram_parameter("bv_d", [DH], F32, isOutput=False)
    Wo_d = nc.declare_dram_parameter("Wo_d", [HID, HID], F16, isOutput=False)
    bo_d = nc.declare_dram_parameter("bo_d", [HID], F32, isOutput=False)
    out_sl = nc.declare_dram_parameter("out_sl", [T_CORE, HID], F32,
                                       isOutput=True)
    sums_dram = nc.dram_tensor("sums_scratch", [NH, T_CORE], F32)

    def bcast_ap(param, n_part):
        a = param[:] if not isinstance(param, bass.AP) else param
        return bass.AP(tensor=a.tensor, offset=a.offset,
                       ap=[[0, n_part]] + list(a.ap))

    from contextlib import ExitStack
    with tile.TileContext(nc, pool_alloc_mode="queue") as tc, ExitStack() as st:
        consts = st.enter_context(tc.tile_pool(name="consts", bufs=1))
        persist = st.enter_context(tc.tile_pool(name="persist", bufs=1))

        ident32 = consts.tile([P, P], F32)
        make_identity(nc, ident32[:])
        ident = consts.tile([P, P], F16)
        nc.vector.tensor_copy(ident[:], ident32[:])
        bq_sb = consts.tile([P, DB], F32)
        nc.sync.dma_start(bq_sb[:], bq_d[:])
        bk_sb = consts.tile([P, 1], F32)
        nc.sync.dma_start(bk_sb[:], bk_d.rearrange("(a one) -> a one", one=1))
        bvb = consts.tile([P, DH], F32)
        nc.gpsimd.dma_start(bvb[:], bcast_ap(bv_d, P))
        bob = consts.tile([P, HID], F32)
        nc.gpsimd.dma_start(bob[:], bcast_ap(bo_d, P))

        # Persistent fp16 operands.
        kT = persist.tile([P, V], F16)             # rows 0-63 K^T, 64-127 dup
        vt = persist.tile([P, NVB, DH + 1], F16)   # V~ + ones col
        qT = persist.tile([P, DB, T_CORE], F16)    # dual-head: even head in
        attnT = persist.tile([P, DB, T_CORE], F16) # parts 0-63, odd in 64-127
        nc.gpsimd.memset(vt[:, :, DH:DH + 1], 1.0)

        # ---- Phase B: K^T and V~ from adj ----
        with (
            tc.tile_pool(name="bin", bufs=3) as bin_p,
            tc.tile_pool(name="baT", bufs=2) as baT_p,
            tc.tile_pool(name="bpsA", bufs=2, space="PSUM") as bpsA,
            tc.tile_pool(name="bpsK", bufs=1, space="PSUM") as bpsK,
            tc.tile_pool(name="bpsV", bufs=1, space="PSUM") as bpsV,
            tc.tile_pool(name="bw1", bufs=1) as bw1,
        ):
            Wk_sb = bw1.tile([P, EB, P], F16)
            nc.sync.dma_start(Wk_sb[:],
                              Wk_d.rearrange("(eb ep) d -> ep eb d", ep=P))
            Wv_sb = bw1.tile([P, EB, DH], F16)
            nc.sync.dma_start(Wv_sb[:],
                              Wv_d.rearrange("(eb ep) d -> ep eb d", ep=P))

            KGRP = 8  # chunks whose K/V projections accumulate before evac
            SUB = 4   # chunks per K-proj batch (N=512 matmuls)
            for cg in range(NCH_V // KGRP):
                pk = bpsK.tile([P, KGRP * P], F32, tag="pk")
                pv = bpsV.tile([P, KGRP, DH], F32, tag="pv")
                for sub in range(KGRP // SUB):
                    aT4 = baT_p.tile([P, EB, SUB * P], F16, tag="aT4")
                    for ci4 in range(SUB):
                        ci = sub * SUB + ci4
                        r0 = (cg * KGRP + ci) * P
                        adj_in = bin_p.tile([P, E], F16, tag="adj_in")
                        nc.sync.dma_start(adj_in[:], adj_b[r0:r0 + P, :])
                        psA = bpsA.tile([P, EB, P], F32, tag="psA")
                        for eb in range(EB):
                            nc.tensor.matmul(psA[:, eb, :],
                                             adj_in[:, eb * P:(eb + 1) * P],
                                             ident[:], start=True, stop=True)
                        nc.scalar.activation(
                            aT4[:, :, ci4 * P:(ci4 + 1) * P], psA[:],
                            mybir.ActivationFunctionType.Copy)
                    for eb in range(EB):
                        nc.tensor.matmul(
                            pk[:, sub * SUB * P:(sub + 1) * SUB * P],
                            Wk_sb[:, eb, :], aT4[:, eb, :],
                            start=(eb == 0), stop=(eb == EB - 1))
                    for ci4 in range(SUB):
                        ci = sub * SUB + ci4
                        for eb in range(EB):
                            nc.tensor.matmul(
                                pv[:, ci, :],
                                aT4[:, eb, ci4 * P:(ci4 + 1) * P],
                                Wv_sb[:, eb, :],
                                start=(eb == 0), stop=(eb == EB - 1))
                # evac: kT with bias (per-partition, fp16 cast on ACT)
                nc.scalar.activation(kT[:, cg * KGRP * P:(cg + 1) * KGRP * P],
                                     pk[:],
                                     mybir.ActivationFunctionType.Identity,
                                     bias=bk_sb[:])
                # evac: V~ with bias (free-dim bias -> DVE tensor add)
                bvb_b = bass.AP(tensor=bvb[:].tensor, offset=bvb[:].offset,
                                ap=[list(bvb[:].ap[0]), [0, KGRP],
                                    list(bvb[:].ap[1])])
                nc.vector.tensor_add(vt[:, cg * KGRP:(cg + 1) * KGRP, 0:DH],
                                     pv[:], bvb_b)

        # ---- Phase C: q^T from x (dual-head layout comes free) ----
        with (
            tc.tile_pool(name="cin", bufs=3) as cin_p,
            tc.tile_pool(name="cxT", bufs=1) as cxT_p,
            tc.tile_pool(name="cpsA", bufs=2, space="PSUM") as cpsA,
            tc.tile_pool(name="cpsQ", bufs=2, space="PSUM") as cpsQ,
            tc.tile_pool(name="cw1", bufs=1) as cw1,
        ):
            Wq_sb = cw1.tile([P, EB, HID], F16)
            nc.sync.dma_start(Wq_sb[:],
                              Wq_d.rearrange("(eb ep) d -> ep eb d", ep=P))
            xT = cxT_p.tile([P, EB, T_CORE], F16)
            for ch in range(NCH_T):
                r0 = ch * P
                x_in = cin_p.tile([P, E], F16, tag="x_in")
                nc.sync.dma_start(x_in[:], x_sl[r0:r0 + P, :])
                psA = cpsA.tile([P, EB, P], F32, tag="psA")
                for eb in range(EB):
                    nc.tensor.matmul(psA[:, eb, :],
                                     x_in[:, eb * P:(eb + 1) * P],
                                     ident[:], start=True, stop=True)
                nc.scalar.activation(xT[:, :, r0:r0 + P], psA[:],
                                     mybir.ActivationFunctionType.Copy)
            for tt in range(NTT):
                ts0 = tt * T_TILE
                for db in range(DB):
                    pq = cpsQ.tile([P, T_TILE], F32, tag="pq")
                    for eb in range(EB):
                        nc.tensor.matmul(
                            pq[:], Wq_sb[:, eb, db * P:(db + 1) * P],
                            xT[:, eb, ts0:ts0 + T_TILE],
                            start=(eb == 0), stop=(eb == EB - 1))
                    nc.scalar.activation(qT[:, db, ts0:ts0 + T_TILE], pq[:],
                                         mybir.ActivationFunctionType.Identity,
                                         bias=bq_sb[:, db:db + 1])

        # ---- Phase D: attention ----
        with (
            tc.tile_pool(name="dP", bufs=7) as dP,
            tc.tile_pool(name="dtmp", bufs=3) as dtmp,
            tc.tile_pool(name="dnorm", bufs=1) as dn,
            tc.tile_pool(name="dpsum", bufs=2, space="PSUM") as dps,
            tc.tile_pool(name="opsum", bufs=1, space="PSUM") as ops,
        ):
            def norm_copy(state):
                O4t, onorm, tt, hg = state
                nc.scalar.activation(onorm[:], O4t[:],
                                     mybir.ActivationFunctionType.Copy)

            def norm_rest(state):
                O4t, onorm, tt, hg = state
                ts0 = tt * T_TILE
                j0 = hg * (HPG // 2)
                h0 = hg * HPG
                nc.gpsimd.dma_start(
                    sums_dram[h0:h0 + HPG, ts0:ts0 + T_TILE],
                    onorm[DH:DH + 1, :, :])
                sbc = dn.tile([DH, HPG, T_TILE], F32, tag="sbc")
                nc.gpsimd.dma_start(
                    sbc[:],
                    bcast_ap(sums_dram[h0:h0 + HPG, ts0:ts0 + T_TILE], DH))
                rec = dn.tile([DH, HPG, T_TILE], F32, tag="rec")
                nc.vector.reciprocal_approx_fast(rec[:], sbc[:])
                for pp in range(HPG // 2):
                    j = j0 + pp
                    nc.vector.tensor_mul(
                        attnT[0:DH, j, ts0:ts0 + T_TILE],
                        onorm[0:DH, 2 * pp, :], rec[:, 2 * pp, :])
                    nrm = dn.tile([DH, T_TILE], F16, tag="nrm%d" % pp)
                    nc.gpsimd.tensor_mul(nrm[:], onorm[0:DH, 2 * pp + 1, :],
                                         rec[:, 2 * pp + 1, :])
                    nc.gpsimd.dma_start(
                        attnT[DH:P, j, ts0:ts0 + T_TILE], nrm[:])

            def emit_pv(item):
                O4t, P2, vb, pp = item
                for h2 in range(2):
                    nc.tensor.matmul(
                        O4t[:, 2 * pp + h2, :], vt[:, vb, :],
                        P2[:, h2 * T_TILE:(h2 + 1) * T_TILE],
                        start=(vb == 0), stop=(vb == NVB - 1),
                        skip_group_check=True)

            prev = None
            pend = []
            for tt in range(NTT):
                ts0 = tt * T_TILE
                for hg in range(NHG):
                    j0 = hg * (HPG // 2)   # dual-head slots j0, j0+1
                    O4t = ops.tile([DH + 1, HPG, T_TILE], F32, tag="O4t")
                    onorm = dn.tile([DH + 1, HPG, T_TILE], F32,
                                    tag="onorm")
                    exp_i = 0
                    for vb in range(NVB):
                        for pp in range(HPG // 2):
                            j = j0 + pp
                            S2 = dps.tile([P, 2 * T_TILE], F32, tag="S2")
                            nc.tensor.matmul(
                                S2[:, 0:T_TILE],
                                kT[0:DH, vb * P:(vb + 1) * P],
                                qT[0:DH, j, ts0:ts0 + T_TILE],
                                start=True, stop=True, tile_position=(0, 0))
                            nc.tensor.matmul(
                                S2[:, T_TILE:2 * T_TILE],
                                kT[DH:P, vb * P:(vb + 1) * P],
                                qT[DH:P, j, ts0:ts0 + T_TILE],
                                start=True, stop=True, tile_position=(64, 0))
                            # previous group's O evacuation goes ahead of
                            # this group's first PV write (slot PV_LAG+1 of
                            # the continuous pend stream).
                            if exp_i == PV_LAG + 1 and prev is not None:
                                norm_copy(prev)
                            P2 = dP.tile([P, 2 * T_TILE], F16, tag="P2")
                            use_dve = (exp_i * DVE_EXP_SHARE) // 64 != \
                                      ((exp_i + 1) * DVE_EXP_SHARE) // 64
                            if use_dve:
                                Ptmp = dtmp.tile([P, 2 * T_TILE], F32,
                                                 tag="Ptmp")
                                nc.vector._custom_dve(
                                    OP_SEED, out=Ptmp[:], in0=S2[:],
                                    s0=EXP_C3, s1=EXP_C2, imm2=EXP_C1)
                                nc.vector._custom_dve(
                                    OP_SQ5, out=P2[:], in0=Ptmp[:])
                            else:
                                nc.scalar.activation(
                                    P2[:], S2[:],
                                    mybir.ActivationFunctionType.Exp,
                                    scale=SCALE)
                            pend.append((O4t, P2, vb, pp))
                            exp_i += 1
                            if exp_i == 10 and prev is not None:
                                norm_rest(prev)
                                prev = None
                            while len(pend) > PV_LAG:
                                emit_pv(pend.pop(0))
                    prev = (O4t, onorm, tt, hg)
            for item in pend:
                emit_pv(item)
            norm_copy(prev)
            norm_rest(prev)

        # ---- Phase E: output projection ----
        with (
            tc.tile_pool(name="ework", bufs=3) as ew,
            tc.tile_pool(name="ew1", bufs=1) as ew1,
            tc.tile_pool(name="epsum", bufs=2, space="PSUM") as ep,
        ):
            Wo_sb = ew1.tile([P, DB, HID], F16)
            nc.sync.dma_start(Wo_sb[:],
                              Wo_d.rearrange("(kb kp) e -> kp kb e", kp=P))
            for tc_i in range(T_CORE // P):
                for eh in range(HID // T_TILE):
                    po = ep.tile([P, T_TILE], F32, tag="po")
                    for kb in range(DB):
                        nc.tensor.matmul(
                            po[:], attnT[:, kb, tc_i * P:(tc_i + 1) * P],
                            Wo_sb[:, kb, eh * T_TILE:(eh + 1) * T_TILE],
                            start=(kb == 0), stop=(kb == DB - 1))
                    ot = ew.tile([P, T_TILE], F32, tag="ot")
                    nc.vector.tensor_add(
                        ot[:], po[:], bob[:, eh * T_TILE:(eh + 1) * T_TILE])
                    nc.sync.dma_start(
                        out_sl[tc_i * P:(tc_i + 1) * P,
                               eh * T_TILE:(eh + 1) * T_TILE], ot[:])

    nc.compile()
    return nc


_NC = None


def _get_nc():
    global _NC
    if _NC is None:
        _NC = build_nc()
    return _NC


def _make_in_maps(inputs):
    x = np.asarray(inputs["x"], np.float32)
    adj = np.asarray(inputs["adj"], np.float32)
    Wq = np.asarray(inputs["Wq"], np.float32)
    bq = np.asarray(inputs["bq"], np.float32)
    Wk_f = np.asarray(inputs["Wk"], np.float32).reshape(E, G, DH).sum(axis=1)
    bk_f = np.asarray(inputs["bk"], np.float32).reshape(G, DH).sum(axis=0)
    Wv_f = np.asarray(inputs["Wv"], np.float32).reshape(E, G, DH).sum(axis=1)
    bv_f = np.asarray(inputs["bv"], np.float32).reshape(G, DH).sum(axis=0)
    Wo = np.asarray(inputs["Wo"], np.float32)
    bo = np.asarray(inputs["bo"], np.float32)

    Wk_dup = np.ascontiguousarray(
        np.concatenate([Wk_f, Wk_f], axis=1).astype(np.float16))  # [E,128]
    bk_dup = np.ascontiguousarray(np.concatenate([bk_f, bk_f]).astype(np.float32))
    bq_dbl = np.ascontiguousarray(bq.reshape(HID // P, P).T.astype(np.float32))

    Wq_16 = np.ascontiguousarray(Wq.astype(np.float16))
    Wv_16 = np.ascontiguousarray(Wv_f.astype(np.float16))
    Wo_16 = np.ascontiguousarray(Wo.astype(np.float16))
    x16 = x.astype(np.float16)
    adj16 = adj.astype(np.float16)

    in_maps = []
    for c in range(N_CORES):
        b = c // (N_CORES // B)
        tq = c % (N_CORES // B)
        in_maps.append({
            "x_sl": np.ascontiguousarray(
                x16[b, tq * T_CORE:(tq + 1) * T_CORE, :]),
            "adj_b": np.ascontiguousarray(adj16[b]),
            "Wq_d": Wq_16, "bq_d": bq_dbl,
            "Wk_d": Wk_dup, "bk_d": bk_dup,
            "Wv_d": Wv_16, "bv_d": np.ascontiguousarray(bv_f),
            "Wo_d": Wo_16, "bo_d": np.ascontiguousarray(bo),
        })
    return in_maps


def kernel(x, adj, Wq, bq, Wk, bk, Wv, bv, Wo, bo):
    inputs = dict(x=x, adj=adj, Wq=Wq, bq=bq, Wk=Wk, bk=bk, Wv=Wv, bv=bv,
                  Wo=Wo, bo=bo)
    nc = _get_nc()
    in_maps = _make_in_maps(inputs)

    from concourse.bass_utils import run_bass_kernel_spmd
    res = run_bass_kernel_spmd(nc, in_maps, list(range(N_CORES)))

    out = np.empty((B, T, HID), np.float32)
    for c in range(N_CORES):
        b = c // (N_CORES // B)
        tq = c % (N_CORES // B)
        out[b, tq * T_CORE:(tq + 1) * T_CORE, :] = res.results[c]["out_sl"]
    return out
